# revision 59
# baseline (speedup 1.0000x reference)
"""Trainium2 Bass kernel for nn_AttPool (4-layer GNN + additive-attention pooling).

Strategy (data-parallel over graphs, 32 graphs per NeuronCore):
  * Host re-lays-out the edge list as per-graph dense normalized adjacency
    Ahat^T = ((A + I) / deg)^T, shipped once in fp8-e4m3 (values lie in
    e4m3's normal range; layer 0 runs it against bf16 h0 as a mixed-dtype
    matmul — quantizing the raw node features themselves is what blows the
    error budget, the adjacency costs ~3e-3 extra).
  * Graphs run in PAIRS, two pairs round-robined per "round" so each
    pair's cast/ACT latencies are covered by the other pair's matmuls.
    Per pair, per layer l:
      - aggT (both graphs into one 2-bank PSUM tile): layer 0 via 4 bf16
        matmuls/graph; layers 1-3 via 2 fp8 DoubleRow matmuls/graph
        (K=256 per pass, both operands fp8, ~2x column throughput).
      - one DVE cast -> bf16, node-major lin (4 matmuls/graph), ONE tanh
        ACT [128,1024] emits node-major x_{l+1} directly in fp8 (short
        critical chain: agg->cast->lin->ACT->next agg).
      - feat-major hT via 8 PE transposes of x (post-tanh, no second
        activation) + one DVE copy; emitted one conv step late so the PE
        queue has independent work between the ACT and its dependents.
  * Attention per graph: uT via 8 fp8 DoubleRow matmuls (attW pre-packed
    x64 in fp8 pairs, descaled inside the tanh ACT's scale); t in two
    fused [128,1024] ACTs; scores via 4 M=1 matmuls accumulating into
    one PSUM row read directly by the exp ACT (no cast, no row-sum);
    exp without max-subtraction (scores are O(1));
    pooling via 16 column-packed matmuls whose 4 partial rows are
    DMA-gathered (unsummed, unnormalized) into a shared [128,D] stack;
    exp-sums accumulate into a shared [1,GPC] tile.
  * Epilogue (once, batched over all graphs): selection matmul sums the
    4 partial rows per graph, 1/expsum lands as the per-partition scale
    of the final ReLU ACT, then transpose + output head matmul.
  lin matmuls write back into the (dead-after-cast) aggT PSUM tile, so
  each round-robined pair owns one conv buffer and never contends.
  Measured end-to-end rel-err vs fp32 reference: 1.39e-2 (gate 2e-2);
  HW exec ~297 us vs 402 us for the bf16 baseline on the same metric.
"""

import numpy as np
import ml_dtypes

B, N, F = 256, 512, 128
NL = 4
D = 512
OUT = 128
NCORES = 8
GPC = B // NCORES  # graphs per core

BF16 = ml_dtypes.bfloat16
FP8 = ml_dtypes.float8_e4m3
ATT_SCALE = 64.0

_NC_CACHE = {}


def _build_nc(has_conv_b, has_att_b, has_out_b):
    key = (has_conv_b, has_att_b, has_out_b)
    if key in _NC_CACHE:
        return _NC_CACHE[key]

    import concourse.bacc as bacc
    import concourse.tile as tile
    import concourse.mybir as mybir
    from concourse.masks import make_identity

    f32 = mybir.dt.float32
    bf16 = mybir.dt.bfloat16
    fp8 = mybir.dt.float8e4
    DR = mybir.MatmulPerfMode.DoubleRow

    nc = bacc.Bacc(None, target_bir_lowering=False)

    gin_d = nc.dram_tensor("gin", [GPC, 128, 4 * D + 2 * F * 4], fp8, kind="ExternalInput")
    convw_d = nc.dram_tensor("convw", [128, NL, F], bf16, kind="ExternalInput")
    attw_d = nc.dram_tensor("attw", [128, 16, F], fp8, kind="ExternalInput")
    attv_d = nc.dram_tensor("attv", [128, 4], bf16, kind="ExternalInput")
    outw_d = nc.dram_tensor("outw", [128, 4 * OUT], bf16, kind="ExternalInput")
    sel_d = nc.dram_tensor("sel", [128, GPC], bf16, kind="ExternalInput")
    out_d = nc.dram_tensor("out", [GPC, OUT], f32, kind="ExternalOutput")
    convb_d = recip_d = attb_d = outb_d = None
    if has_conv_b:
        convb_d = nc.dram_tensor("convb", [1, NL * F], f32, kind="ExternalInput")
        recip_d = nc.dram_tensor("recipdeg", [GPC, D], f32, kind="ExternalInput")
    if has_att_b:
        attb_d = nc.dram_tensor("attb", [128, 4], f32, kind="ExternalInput")
    if has_out_b:
        outb_d = nc.dram_tensor("outb", [1, OUT], f32, kind="ExternalInput")

    with tile.TileContext(nc) as tc:
        with (
            tc.tile_pool(name="singles", bufs=1) as singles,
        ):
            convw_sb = singles.tile([128, NL, F], bf16)
            attw_sb = singles.tile([128, 16, F], fp8)
            attv_sb = singles.tile([128, 4], bf16)
            outw_sb = singles.tile([128, 4 * OUT], bf16)
            ident = singles.tile([128, 128], fp8)
            make_identity(nc, ident[:])
            ident32 = singles.tile([32, 32], bf16)
            make_identity(nc, ident32[:])
            one1 = singles.tile([1, 1], bf16)
            nc.vector.memset(one1[:], 1.0)
            one1f = singles.tile([1, 1], f32)
            nc.vector.memset(one1f[:], 1.0)
            convb_sb = attb_sb = outb_sb = ones_sb = None
            if has_conv_b:
                convb_sb = singles.tile([1, NL * F], f32)
                nc.sync.dma_start(convb_sb[:], convb_d[:])
            if has_att_b:
                attb_sb = singles.tile([128, 4], f32)
                nc.sync.dma_start(attb_sb[:], attb_d[:])
            if has_out_b:
                outb_sb = singles.tile([1, OUT], f32)
                nc.sync.dma_start(outb_sb[:], outb_d[:])
                ones_sb = singles.tile([1, 32], f32)
                nc.vector.memset(ones_sb[:], 1.0)

            # keep later singles tiles 4-byte aligned (a [128,1] bf16 tile
            # used to sit here; removing it shifted f32 tiles to 2-byte
            # offsets and cost ~55us)
            pad2 = singles.tile([128, 1], bf16)
            # per-graph pooled4 partial rows (4 per graph, f32) gathered by
            # DMA; summed once in Phase B via a selection matmul
            pstack4 = singles.tile([128, D], bf16)
            ssum_stack = singles.tile([1, GPC], f32)
            sel_sb = singles.tile([128, GPC], bf16)
            nc.sync.dma_start(sel_sb[:], sel_d[:])

            # ---------------- Phase A: convs + attention ----------------
            with (
                tc.tile_pool(name="gin", bufs=8) as p_gin,
                tc.tile_pool(name="x", bufs=4 * NL + 4) as p_x,
                tc.tile_pool(name="hT", bufs=4) as p_hT,
                tc.tile_pool(name="agg", bufs=6) as p_agg,
                tc.tile_pool(name="t", bufs=12) as p_t,
                tc.tile_pool(name="sm", bufs=12) as p_sm,
                tc.tile_pool(name="rc", bufs=8) as p_rc,
                tc.tile_pool(name="ps_conv", bufs=2, space="PSUM") as ps_conv,
                tc.tile_pool(name="ps_tp", bufs=1, space="PSUM") as ps_tp,
                tc.tile_pool(name="ps_small", bufs=1, space="PSUM") as ps_small,
                tc.tile_pool(name="ps_uT", bufs=1, space="PSUM") as ps_uT,
            ):
                xs = {}
                h0s = {}
                hTs = {}
                recips = {}
                at8_sbs = {}

                def issue_dma(gg, split_first):
                    gin_t = p_gin.tile([128, 4 * D + 2 * F * 4], fp8, tag="gin")
                    at8_sbs[gg] = gin_t[:, : 4 * D].rearrange(
                        "p (c n) -> p c n", c=4
                    )
                    h0s[gg] = (
                        gin_t[:, 4 * D :].bitcast(bf16).rearrange(
                            "p (c n) -> p c n", c=4
                        )
                    )
                    if split_first:
                        # split the first graph's load so the first matmul
                        # starts as soon as h0 + adjacency chunk 0 land
                        nc.sync.dma_start(gin_t[:, 4 * D :], gin_d[gg, :, 4 * D :])
                        for c in range(4):
                            nc.sync.dma_start(
                                gin_t[:, c * D : (c + 1) * D],
                                gin_d[gg, :, c * D : (c + 1) * D],
                            )
                    else:
                        nc.sync.dma_start(gin_t[:], gin_d[gg])
                    if has_conv_b:
                        rc_t = p_rc.tile([1, D], f32)
                        recips[gg] = rc_t
                        nc.sync.dma_start(rc_t[:], recip_d[gg : gg + 1, :])

                astate = {}

                def conv_pair_step(pair, l):
                    # both graphs' aggT in one [128,1024] PSUM tile (2 banks)
                    agg_ps = ps_conv.tile([128, 2, D], f32, tag="conv")
                    for g01, gg in enumerate(pair):
                        if l == 0:
                            for c in range(4):
                                nc.tensor.matmul(
                                    agg_ps[:, g01, :],
                                    h0s[gg][:, c, :],
                                    at8_sbs[gg][:, c, :],
                                    start=(c == 0),
                                    stop=(c == 3),
                                )
                        else:
                            xp, xg = xs[(gg, l)]
                            for p in range(2):
                                nc.tensor.matmul(
                                    agg_ps[:, g01, :],
                                    xp[:, xg, 2 * p : 2 * p + 2, :],
                                    at8_sbs[gg][:, 2 * p : 2 * p + 2, :],
                                    start=(p == 0),
                                    stop=(p == 1),
                                    perf_mode=DR,
                                )
                    agg_sb = p_agg.tile([128, 2, D], bf16, tag="agg")
                    nc.vector.tensor_copy(agg_sb[:], agg_ps[:])

                    # node-major lin for BOTH graphs: x_{l+1} comes straight
                    # off the tanh ACT (short critical chain); the feat-major
                    # hT copy happens off-chain via PE transposes below
                    # lin reuses the agg PSUM tile (agg is dead after the
                    # cast; WAR ordering via the tile tracker is exactly the
                    # conv chain) — each pair owns one conv buffer, so the two
                    # round-robined pairs never contend on the ring
                    lin_ps = agg_ps
                    for g01, gg in enumerate(pair):
                        for r in range(4):
                            o = lin_ps[:, g01, r * F : (r + 1) * F]
                            if has_conv_b:
                                nc.tensor.matmul(
                                    o,
                                    recips[gg][0:1, r * F : (r + 1) * F],
                                    convb_sb[0:1, l * F : (l + 1) * F],
                                    start=True,
                                    stop=False,
                                )
                            nc.tensor.matmul(
                                o,
                                agg_sb[:, g01, r * F : (r + 1) * F],
                                convw_sb[:, l, :],
                                start=not has_conv_b,
                                stop=True,
                            )
                    x_pair = p_x.tile([128, 2, 4, F], fp8, tag="x")
                    for g01, gg in enumerate(pair):
                        xs[(gg, l + 1)] = (x_pair, g01)
                    nc.scalar.activation(
                        x_pair[:],
                        lin_ps[:],
                        mybir.ActivationFunctionType.Tanh,
                    )
                    if l == 0:
                        hT_pair = p_hT.tile([128, NL, 2, D], fp8, tag="hT")
                        hTs[pair[0]] = hT_pair

                def emit_hT(pair, l):
                    # feat-major hT via PE transposes of x (post-tanh, fp8
                    # transpose mode requires output element step 2).  Emitted
                    # one conv step late so the PE queue has independent work
                    # between the x-ACT and these dependent transposes.
                    hT_pair = hTs[pair[0]]
                    x_pair, _ = xs[(pair[0], l + 1)]
                    tp_ps = ps_tp.tile([128, 2, 4, F, 2], fp8, tag="tp")
                    for g01 in range(2):
                        for c in range(4):
                            nc.tensor.transpose(
                                tp_ps[:, g01, c, :, 0],
                                x_pair[:, g01, c, :],
                                ident[:],
                            )
                    nc.vector.tensor_copy(
                        hT_pair[:, l, :, :],
                        tp_ps[:, :, :, :, 0].rearrange("p g c n -> p g (c n)"),
                    )

                def attn_graph(pair, g01):
                    gg = pair[g01]
                    hT_pair = hTs[pair[0]]
                    t_sbs = []
                    for mh in range(2):  # m half: (m=2mh, 2mh+1)
                        uT_ps = ps_uT.tile([128, 2, D], f32, tag="uT")
                        for mi in range(2):
                            m = 2 * mh + mi
                            for p in range(2):
                                nc.tensor.matmul(
                                    uT_ps[:, mi, :],
                                    attw_sb[:, 4 * m + 2 * p : 4 * m + 2 * p + 2, :],
                                    hT_pair[:, 2 * p : 2 * p + 2, g01, :],
                                    start=(p == 0),
                                    stop=(p == 1),
                                    perf_mode=DR,
                                )
                        t_sb = p_t.tile([128, 2, D], bf16, tag="t")
                        if has_att_b:
                            for mi in range(2):
                                m = 2 * mh + mi
                                nc.scalar.activation(
                                    t_sb[:, mi, :],
                                    uT_ps[:, mi, :],
                                    mybir.ActivationFunctionType.Tanh,
                                    bias=attb_sb[:, m : m + 1],
                                    scale=1.0 / ATT_SCALE,
                                )
                        else:
                            nc.scalar.activation(
                                t_sb[:],
                                uT_ps[:],
                                mybir.ActivationFunctionType.Tanh,
                                scale=1.0 / ATT_SCALE,
                            )
                        t_sbs.append(t_sb)
                    # scores: 4 concurrent M=1 matmuls on distinct column
                    # 4 M=1 matmuls accumulating into one PSUM row: the
                    # exp-ACT reads it directly (no cast, no mask row-sum)
                    s_ps = ps_small.tile([1, D], f32, tag="small")
                    for m in range(4):
                        nc.tensor.matmul(
                            s_ps[:],
                            attv_sb[:, m : m + 1],
                            t_sbs[m // 2][:, m % 2, :],
                            start=(m == 0),
                            stop=(m == 3),
                        )
                    # scores are O(1); exp without max-subtraction is safe.
                    # exp-sums accumulate straight into a shared [1,GPC] tile;
                    # normalization is folded into Phase B's ReLU scale.
                    attn_u = p_sm.tile([1, D], bf16, tag="attnu")
                    nc.scalar.activation(
                        attn_u[:],
                        s_ps[:],
                        mybir.ActivationFunctionType.Exp,
                        accum_out=ssum_stack[0:1, gg : gg + 1],
                    )
                    # attn column extraction: [1,512] -> [128,4]
                    col_ps = ps_small.tile([128, 4], f32, tag="small")
                    for r in range(4):
                        nc.tensor.matmul(
                            col_ps[:, r : r + 1],
                            attn_u[0:1, r * 128 : (r + 1) * 128],
                            one1[:],
                            start=(r == 0),
                            stop=(r == 3),
                        )
                    attn_col = p_sm.tile([128, 4], bf16, tag="acol")
                    nc.vector.tensor_copy(attn_col[:], col_ps[:])
                    # pooling: 16 column-packed matmuls
                    pooled4_ps = ps_small.tile([128, D], f32, tag="small")
                    for l in range(NL):
                        xp, xg = xs[(gg, l + 1)]
                        for r in range(4):
                            nc.tensor.matmul(
                                pooled4_ps[32 * r : 32 * r + 1, l * F : (l + 1) * F],
                                attn_col[:, r : r + 1],
                                xp[:, xg, r, :],
                                start=(l == 0),
                                stop=(l == 3),
                                tile_position=(0, 32 * r),
                            )
                    pooled4_sb = p_t.tile([128, D], bf16, tag="s4")
                    nc.vector.tensor_copy(pooled4_sb[:], pooled4_ps[:])
                    nc.sync.dma_start(
                        pstack4[4 * gg : 4 * gg + 4, :], pooled4_sb[0:128:32, :]
                    )

                RND = 4
                for rp in range(0, GPC, RND):
                    pairs = [(g, g + 1) for g in range(rp, min(rp + RND, GPC), 2)]
                    if rp == 0:
                        issue_dma(0, True)
                        for gg in range(1, RND):
                            issue_dma(gg, False)
                        nc.sync.dma_start(convw_sb[:], convw_d[:])
                        nc.sync.dma_start(attw_sb[:], attw_d[:])
                        nc.sync.dma_start(attv_sb[:], attv_d[:])
                        nc.sync.dma_start(outw_sb[:], outw_d[:])
                    # prefetch the NEXT round's tensors while this one computes
                    for gg in range(rp + RND, min(rp + 2 * RND, GPC)):
                        issue_dma(gg, False)
                    # round-robin the two pairs through the layer loop: each
                    # pair's cast/ACT waits are covered by the other pair's
                    # matmuls, with no extra PSUM footprint
                    for l in range(NL):
                        for pr in pairs:
                            conv_pair_step(pr, l)
                            if l >= 1:
                                emit_hT(pr, l - 1)
                    for pr in pairs:
                        emit_hT(pr, NL - 1)
                    for g01 in range(2):
                        for pr in pairs:
                            attn_graph(pr, g01)

            # ---------------- Phase B: output head --------------------
            with (
                tc.tile_pool(name="fin", bufs=2) as p_fin,
                tc.tile_pool(name="ps_tp", bufs=2, space="PSUM") as ps_tp,
                tc.tile_pool(name="ps_out", bufs=1, space="PSUM") as ps_out,
            ):
                pooled_ps = ps_out.tile([GPC, D], f32)
                nc.tensor.matmul(
                    pooled_ps[:], sel_sb[:], pstack4[:], start=True, stop=True
                )
                pooled_sb = p_fin.tile([GPC, D], bf16)
                nc.vector.tensor_copy(pooled_sb[:], pooled_ps[:])
                recip_row = p_fin.tile([1, GPC], f32)
                nc.vector.reciprocal(recip_row[:], ssum_stack[:])
                rc_ps = ps_tp.tile([GPC, 1], f32)
                nc.tensor.matmul(rc_ps[:], recip_row[:], one1f[:], start=True, stop=True)
                recip_col = p_fin.tile([GPC, 1], f32)
                nc.vector.tensor_copy(recip_col[:], rc_ps[:])
                pT = p_fin.tile([128, 4 * GPC], bf16)
                for c in range(4):
                    tp_ps = ps_tp.tile([128, GPC], bf16)
                    nc.tensor.transpose(
                        tp_ps[:], pooled_sb[:, c * 128 : (c + 1) * 128], ident32[:]
                    )
                    nc.scalar.copy(pT[:, c * GPC : (c + 1) * GPC], tp_ps[:])
                out_ps = ps_out.tile([GPC, OUT], f32)
                if has_out_b:
                    nc.tensor.matmul(
                        out_ps[:], ones_sb[:], outb_sb[:], start=True, stop=False
                    )
                for c in range(4):
                    nc.tensor.matmul(
                        out_ps[:],
                        pT[:, c * GPC : (c + 1) * GPC],
                        outw_sb[:, c * OUT : (c + 1) * OUT],
                        start=(c == 0 and not has_out_b),
                        stop=(c == 3),
                    )
                out_sb = p_fin.tile([GPC, OUT], f32)
                nc.scalar.activation(
                    out_sb[:],
                    out_ps[:],
                    mybir.ActivationFunctionType.Relu,
                    scale=recip_col[:, 0:1],
                )
                nc.sync.dma_start(out_d[:], out_sb[:])

    nc.compile()
    _NC_CACHE[key] = nc
    return nc


def _prep_inputs(node_feat, edge_src, edge_dst, conv_W, att_W, att_v, out_W):
    src = edge_src.astype(np.int64)
    dst = edge_dst.astype(np.int64)
    ls = src - (dst // N) * N  # src local id within dst's graph
    idx = dst * N + ls
    counts = np.bincount(idx, minlength=B * N * N).astype(np.float32)
    A = counts.reshape(B, N, N)
    iN = np.arange(N)
    A[:, iN, iN] += 1.0
    degs = A.sum(axis=2)  # == deg + 1
    Ahat = A / degs[:, :, None]
    At = np.ascontiguousarray(Ahat.transpose(0, 2, 1))  # [g, src, dst]
    at_host = np.ascontiguousarray(At.reshape(B, 4, 128, N).transpose(0, 2, 1, 3))
    # [B, 128, 4, N]

    h0_host = np.ascontiguousarray(
        node_feat.reshape(B, 4, 128, F).transpose(0, 2, 1, 3)
    )  # [B, 128, 4, F]

    convw2 = np.ascontiguousarray(conv_W.transpose(1, 0, 2))  # [128, NL, F]
    # attw DoubleRow packing: [128, 16, 128] where index m*4 + 2p + j holds
    # att_W rows (2p+j)*128:(2p+j+1)*128, cols m*128:(m+1)*128
    attw_dr = np.ascontiguousarray(
        att_W.reshape(4, 128, 4, 128).transpose(1, 0, 2, 3)  # [128, lc, m, 128]
    )
    attw2 = np.empty((128, 16, 128), dtype=np.float32)
    for m in range(4):
        for p in range(2):
            for j in range(2):
                attw2[:, 4 * m + 2 * p + j, :] = attw_dr[:, 2 * p + j, m, :]
    attv2 = np.ascontiguousarray(att_v.reshape(4, 128).T)
    outw2 = np.ascontiguousarray(
        out_W.reshape(4, 128, OUT).transpose(1, 0, 2)
    ).reshape(128, 4 * OUT)
    return at_host, h0_host, convw2, attw2, attv2, outw2, degs


def kernel(
    node_feat,
    edge_src,
    edge_dst,
    conv_W,
    conv_b,
    att_W,
    att_b,
    att_v,
    out_W,
    out_b,
):
    from concourse.bass_utils import run_bass_kernel_spmd

    at_host, h0_host, convw2, attw2, attv2, outw2, degs = _prep_inputs(
        np.asarray(node_feat, dtype=np.float32),
        np.asarray(edge_src),
        np.asarray(edge_dst),
        np.asarray(conv_W, dtype=np.float32),
        np.asarray(att_W, dtype=np.float32),
        np.asarray(att_v, dtype=np.float32),
        np.asarray(out_W, dtype=np.float32),
    )
    conv_b = np.asarray(conv_b, dtype=np.float32)
    att_b = np.asarray(att_b, dtype=np.float32)
    out_b = np.asarray(out_b, dtype=np.float32)
    has_conv_b = bool(np.any(conv_b))
    has_att_b = bool(np.any(att_b))
    has_out_b = bool(np.any(out_b))

    nc = _build_nc(has_conv_b, has_att_b, has_out_b)

    at8_h = at_host.astype(FP8)
    h0_b = h0_host.astype(BF16)
    gin_h = np.concatenate(
        [
            at8_h.view(np.uint8).reshape(B, 128, 4 * D),
            h0_b.view(np.uint8).reshape(B, 128, 2 * 4 * F),
        ],
        axis=2,
    ).view(FP8)
    convw_b = convw2.astype(BF16)
    attw_b = (attw2 * ATT_SCALE).astype(FP8)
    attv_b = attv2.astype(BF16)
    sel_h = np.zeros((128, GPC), dtype=BF16)
    for g in range(GPC):
        sel_h[4 * g : 4 * g + 4, g] = 1.0
    outw_b = outw2.astype(BF16)

    in_maps = []
    for c in range(NCORES):
        sl = slice(c * GPC, (c + 1) * GPC)
        m = {
            "gin": gin_h[sl],
            "convw": convw_b,
            "attw": attw_b,
            "attv": attv_b,
            "sel": sel_h,
            "outw": outw_b,
        }
        if has_conv_b:
            m["convb"] = conv_b.reshape(1, NL * F)
            m["recipdeg"] = (1.0 / degs[sl]).astype(np.float32)
        if has_att_b:
            m["attb"] = np.ascontiguousarray(att_b.reshape(4, 128).T)
        if has_out_b:
            m["outb"] = out_b.reshape(1, OUT)
        in_maps.append(m)

    res = run_bass_kernel_spmd(nc, in_maps, core_ids=list(range(NCORES)))
    out = np.concatenate([r["out"] for r in res.results], axis=0)
    return np.ascontiguousarray(out.astype(np.float32))
